# revision 34
# baseline (speedup 1.0000x reference)
"""Trainium2 Bass kernel for nn_CausalSelfAttention_2224793059575.

Tensor-parallel over heads across 8 NeuronCores: core c owns head c
(B=1, T=2048, D=1024, H=8, HD=128). Per core:

  - QKV projection (contraction over D) consumes x^T (host-prepared layout,
    bf16) against per-head weight slices, emitting q/k in a transposed
    [head_dim, T] layout stacked as A=[q_lo;k_lo], B=[q_hi;k_hi] so that
    RMS-norm scaling and RoPE run as full-128-partition DVE ops.
  - Each 512-col chunk is split into a matmul-only part (p1_mm: QKV, gate
    logits, sum-of-squares) and a dependent finish (p1_fin: rsqrt chain,
    RMS scale, RoPE written straight into qT/kT halves, v transpose), so
    the PE never sits behind the ACT/DVE chain: p1_fin(ch) is emitted
    after p1_mm(ch+1).
  - The QKV PSUM accumulators are copied to SBUF (bf16) immediately, so
    two PSUM banks suffice for double-buffered QKV.
  - rsqrt via Ln/Exp on ScalarE. All ACT functions used (exp/ln/square/
    copy) live in the natural_log_exp_and_others table set; the table list
    handed to the load-insertion pass is filtered so that set is the only
    candidate, giving exactly one ACT_TABLE_LOAD for the whole kernel
    (walrus otherwise alternates exp_and_others <-> natural_log, a 3us
    reload per chunk on the critical path).
  - Scores are computed transposed (S^T[k,q]); exp on ScalarE; causal
    masking only of the 128x128 diagonal block; softmax denominator via an
    all-ones [128,128] lhsT so the row-sums arrive pre-broadcast over all
    partitions; the head-gate logits use a host-tiled [128,128] gw so the
    sigmoid input is also pre-broadcast. The normalization
    y' = y / ((1+e^-g) * l) is then 3 wide DVE ops, no PE broadcasts.
  - y^T chunks are exchanged with four 128KB AllToAlls, one per chunk,
    triggered as soon as each chunk's y^T is finalized. The entire exchange
    pipeline (stage DMA -> collective -> land DMA) lives on the gpsimd DMA
    ring: collective n+1 already waits for collective n's completion there,
    so the land triggers add no blocking and fire the moment their exchange
    completes (the sync ring's strict FIFO interleaves Tile semaphore ops
    that would stall them for tens of us). A full-size garbage AllToAll on
    the same buffers runs during the compute fill phase to absorb the
    kernel-start skew and the ~20-25us first-collective channel-wake cost,
    after which each real exchange takes ~8-14us. The output projection for
    rows 0..127 and chunk 2 runs inside the later exchanges' wait windows;
    chunk 3's rows follow the last landing. Host reassembles the
    interleaved row blocks.

Sharding/layout prep (slicing qkvo_w per head, transposes, bf16 casts,
folding sa_lambdas into the weight slices) happens host-side in numpy, as
input preparation; all FLOPs of the module run on the NeuronCores.
"""
import contextlib
import ctypes
import os
import sys
import types

import numpy as np

for _p in ("/opt/trn_rl_repo",):
    if _p not in sys.path:
        sys.path.append(_p)

import ml_dtypes  # noqa: E402

import concourse.bacc as bacc  # noqa: E402
import concourse.mybir as mybir  # noqa: E402
import concourse.tile as tile  # noqa: E402
from concourse import bass_utils  # noqa: E402
from concourse.tile_rust import add_dep_helper  # noqa: E402

BF16 = mybir.dt.bfloat16
FP32 = mybir.dt.float32
AF = mybir.ActivationFunctionType
OP = mybir.AluOpType

N_CORES = 8
T = 2048
D = 1024
H = 8
HD = 128
HALF = HD // 2  # 64
NCH = 4          # T chunks of 512
CH = T // NCH    # 512
KT = T // 128    # 16 k-tiles
BLK = CH // N_CORES  # 64-wide t-blocks for the interleaved A2A sharding
ATTN_SCALE = 0.12
EPS = 1e-6
GATE_IN = 12

LAST_RUN_INFO = {}


@contextlib.contextmanager
def _single_act_table():
    """Restrict the ACT table-set candidates to natural_log_exp_and_others.

    The set genuinely contains every function this kernel uses (exp, ln,
    square, copy/identity), but the load-insertion pass otherwise picks the
    first set containing each function (exp_and_others for exp, natural_log
    for ln), forcing a ~1.5us table reload at every ln<->exp transition.
    Names and positions are preserved so the emitted act_func_set_id still
    matches act_info.json.
    """
    orig = bacc.get_activation_tables

    def patched(arch):
        tabs = orig(arch)
        return {
            name: (fns if name == "natural_log_exp_and_others" else set())
            for name, fns in tabs.items()
        }

    bacc.get_activation_tables = patched
    try:
        yield
    finally:
        bacc.get_activation_tables = orig


def _build_program():
    nc = bacc.Bacc("TRN2", target_bir_lowering=False, debug=False,
                   num_devices=N_CORES)
    ctx_tables = _single_act_table()
    ctx_tables.__enter__()

    # ---- kernel I/O ----
    xT_d = nc.dram_tensor("xT", [D, T], BF16, kind="ExternalInput")
    wA_d = nc.dram_tensor("wA", [128, 8 * 128], BF16, kind="ExternalInput")
    wB_d = nc.dram_tensor("wB", [128, 8 * 128], BF16, kind="ExternalInput")
    wV_d = nc.dram_tensor("wV", [128, 8 * 128], BF16, kind="ExternalInput")
    wO_d = nc.dram_tensor("wO", [128, 8 * D], BF16, kind="ExternalInput")
    c2_d = nc.dram_tensor("c2", [128, T], BF16, kind="ExternalInput")
    s2_d = nc.dram_tensor("s2", [128, T], BF16, kind="ExternalInput")
    ve_d = nc.dram_tensor("ve_s", [128, KT * HD], BF16, kind="ExternalInput")
    gw_d = nc.dram_tensor("gw", [128, 128], BF16, kind="ExternalInput")
    out_d = nc.dram_tensor("out_t", [T // N_CORES, D], FP32, kind="ExternalOutput")

    with tile.TileContext(nc) as tc, contextlib.ExitStack() as ctx:
        P = ctx.enter_context

        cons = P(tc.tile_pool(name="cons", bufs=1))
        work = P(tc.tile_pool(name="work", bufs=1))
        absp = P(tc.tile_pool(name="absp", bufs=4))
        sqp = P(tc.tile_pool(name="sqp", bufs=4))
        rowp = P(tc.tile_pool(name="rowp", bufs=2))
        rbp = P(tc.tile_pool(name="rbp", bufs=2))
        tmp = P(tc.tile_pool(name="tmp", bufs=8))
        egp = P(tc.tile_pool(name="egp", bufs=3))
        ptp = P(tc.tile_pool(name="ptp", bufs=14))
        denp = P(tc.tile_pool(name="denp", bufs=3))
        outp = P(tc.tile_pool(name="outp", bufs=2))
        dram = P(tc.tile_pool(name="dram", bufs=1, space="DRAM"))

        # PSUM: 8 banks total, statically budgeted
        psAB = P(tc.tile_pool(name="psAB", bufs=2, space="PSUM"))
        psS = P(tc.tile_pool(name="psS", bufs=2, space="PSUM"))
        psY = P(tc.tile_pool(name="psY", bufs=1, space="PSUM"))
        psBB = P(tc.tile_pool(name="psBB", bufs=1, space="PSUM"))
        psM = P(tc.tile_pool(name="psM", bufs=1, space="PSUM"))
        psL = P(tc.tile_pool(name="psL", bufs=1, space="PSUM"))

        # ---- persistent SBUF ----
        xT = cons.tile([128, 8, T], BF16)          # x^T, i-tile major
        wA = cons.tile([128, 8, 128], BF16)
        wB = cons.tile([128, 8, 128], BF16)
        wV = cons.tile([128, 8, 128], BF16)
        wO = cons.tile([128, 8, D], BF16)
        c2 = cons.tile([128, T], BF16)
        s2 = cons.tile([128, T], BF16)
        ve = cons.tile([128, KT, HD], BF16)
        gw = cons.tile([128, 128], BF16)
        ones = cons.tile([128, 1], BF16)
        ones128 = cons.tile([128, 128], BF16)
        ones1 = cons.tile([33, 128], BF16)
        sel33 = cons.tile([128, 33], BF16)
        ident = cons.tile([128, 128], BF16)
        tri = cons.tile([128, 128], BF16)
        eps_c = cons.tile([128, 1], FP32)
        wup = cons.tile([128, 512], BF16)  # zeroed warmup operand

        qT = work.tile([128, T], BF16)
        kT_t = work.tile([128, T], BF16)
        vT = work.tile([128, T], BF16)
        v_sb = work.tile([128, KT, HD], BF16)
        yT = work.tile([128, T], BF16)
        # [j-tile, ch, 64] free layout; [:, j, 2p:2p+2, :] is a contiguous
        # 128-wide lhsT slice for the output projection
        ygT = work.tile([128, 8, NCH, BLK], BF16)

        a2a_in = [dram.tile([D, BLK], BF16, name=f"a2ain{i}") for i in range(NCH)]
        a2a_out = [dram.tile([D, BLK], BF16, name=f"a2aout{i}") for i in range(NCH)]
        bar_in = dram.tile([8, 64], BF16, name="barin")
        bar_out = dram.tile([8, 64], BF16, name="barout")

        # PE warmup first thing: result is never read -- it exists purely to
        # get HAM to 8/8 before the first QKV matmul. wup is zeroed on the
        # otherwise-idle DVE so the warmup doesn't wait for gpsimd's memsets.
        nc.vector.memset(wup[:], 0.0)
        pwu = psS.tile([128, CH], FP32, tag="pS")
        # N=512 warmup matmuls: ~6-9us of sustained PE activity so the HAM
        # clock-gate stays at 8/8 across the input-DMA window until the first
        # QKV matmul is ready (a >3.4us idle gap re-throttles to 4/8).
        for i in range(28):
            nc.tensor.matmul(pwu[0:1, :], wup[:, 0:1], wup[:],
                             start=(i == 0), stop=(i == 27),
                             skip_group_check=True)

        # ---- on-chip constants + priority-ordered input DMAs ----
        nc.gpsimd.memset(ones[:], 1.0)
        nc.gpsimd.memset(ones128[:], 1.0)
        nc.gpsimd.memset(ones1[:], 1.0)
        nc.gpsimd.memset(sel33[:], 0.0)
        nc.gpsimd.memset(sel33[0:64, 0:1], 1.0)
        nc.gpsimd.memset(sel33[64:128, 32:33], 1.0)
        nc.gpsimd.memset(ident[:], 1.0)
        nc.gpsimd.memset(eps_c[:], EPS)
        # keep 1.0 where p == f, else 0
        nc.gpsimd.affine_select(out=ident[:], in_=ident[:], compare_op=OP.is_equal,
                                fill=0.0, base=0, pattern=[[-1, 128]],
                                channel_multiplier=1)
        # upper-triangular (incl. diagonal) ones: keep where f - p >= 0
        nc.gpsimd.memset(tri[:], 1.0)
        nc.gpsimd.affine_select(out=tri[:], in_=tri[:], compare_op=OP.is_ge,
                                fill=0.0, base=0, pattern=[[1, 128]],
                                channel_multiplier=-1)

        # Warm the collective path end-to-end with a full-size garbage
        # exchange on the real buffers: the first data-size collective of a
        # run otherwise pays ~20-25us of one-time channel wake/init (a tiny
        # 1KB barrier does not warm the same channels). This completes during
        # the compute fill phase, before stage(0) needs the gpsimd queue.
        nc.gpsimd.collective_compute(
            "AllToAll", OP.bypass,
            replica_groups=[list(range(N_CORES))],
            ins=[a2a_in[0][:].opt()], outs=[a2a_out[0][:].opt()])

        nc.sync.dma_start(wA[:], wA_d[:].rearrange("p (i e) -> p i e", i=8))
        for i in range(8):
            nc.sync.dma_start(xT[:, i, 0:CH], xT_d[i * 128:(i + 1) * 128, 0:CH])
        nc.sync.dma_start(wB[:], wB_d[:].rearrange("p (i e) -> p i e", i=8))
        nc.sync.dma_start(wV[:], wV_d[:].rearrange("p (i e) -> p i e", i=8))
        nc.sync.dma_start(gw[:], gw_d[:])
        for i in range(8):
            nc.sync.dma_start(xT[:, i, CH:2 * CH],
                              xT_d[i * 128:(i + 1) * 128, CH:2 * CH])
        for i in range(8):
            nc.sync.dma_start(xT[:, i, 2 * CH:T],
                              xT_d[i * 128:(i + 1) * 128, 2 * CH:T])
        nc.sync.dma_start(c2[:], c2_d[:])
        nc.sync.dma_start(s2[:], s2_d[:])
        nc.sync.dma_start(ve[:], ve_d[:].rearrange("p (k d) -> p k d", k=KT))

        # per-chunk state handed from p1_mm to p1_fin / phase2
        st = {}

        def p1_mm(ch):
            sl = slice(ch * CH, (ch + 1) * CH)
            s = {}
            # QKV (A/B stacked halves of q,k; v transposed), N=512 matmuls
            pA = psAB.tile([128, CH], FP32, tag="pAB")
            pB = psAB.tile([128, CH], FP32, tag="pAB")
            for i in range(8):
                nc.tensor.matmul(pA[:], wA[:, i, :], xT[:, i, sl],
                                 start=(i == 0), stop=(i == 7))
            # free the PSUM bank quickly: bf16 copy on DVE, square on ACT
            # (parallel engines keep the rsqrt chain short)
            A_s = absp.tile([128, CH], BF16, tag="ab_sb")
            sqA = sqp.tile([128, CH], BF16, tag="sq")
            with tc.high_priority(offset=1200):
                nc.vector.tensor_copy(out=A_s[:], in_=pA[:])
            nc.scalar.activation(sqA[:], pA[:], AF.Square)

            # gate logits, broadcast over all partitions via host-tiled gw;
            # emitted between the A and B blocks so e_g's ACT read of the psM
            # bank finishes under the B matmuls (pVt reuses that bank)
            pg = psM.tile([128, CH], FP32, tag="m")
            nc.tensor.matmul(pg[:], gw[:], xT[:, 0, sl], start=True, stop=True)
            e_g = egp.tile([128, CH], BF16, tag="eg")
            nc.scalar.activation(e_g[:], pg[:], AF.Exp, scale=-1.0)
            s["e_g"] = e_g

            for i in range(8):
                nc.tensor.matmul(pB[:], wB[:, i, :], xT[:, i, sl],
                                 start=(i == 0), stop=(i == 7))
            B_s = absp.tile([128, CH], BF16, tag="ab_sb")
            sqB = sqp.tile([128, CH], BF16, tag="sq")
            with tc.high_priority(offset=1200):
                nc.vector.tensor_copy(out=B_s[:], in_=pB[:])
            nc.scalar.activation(sqB[:], pB[:], AF.Square)

            pVt = psM.tile([128, CH], FP32, tag="m")
            for i in range(8):
                nc.tensor.matmul(pVt[:], wV[:, i, :], xT[:, i, sl],
                                 start=(i == 0), stop=(i == 7))
            nc.vector.tensor_copy(out=vT[:, sl], in_=pVt[:])

            # sum of squares -> [33, CH] rows (q at partition 0; k at 32)
            pssq = psBB.tile([33, CH], FP32, tag="bb")
            nc.tensor.matmul(pssq[:], sel33[:], sqA[:], start=True, stop=False)
            nc.tensor.matmul(pssq[:], sel33[:], sqB[:], start=False, stop=True)
            s["pssq"] = pssq
            s["A_s"] = A_s
            s["B_s"] = B_s
            st[ch] = s

        def p1_fin(ch):
            sl = slice(ch * CH, (ch + 1) * CH)
            s = st[ch]
            # rsqrt rows via Ln/Exp (one shared ACT table set)
            lssq = rowp.tile([33, CH], FP32, tag="row")
            rinv_b = rowp.tile([33, CH], BF16, tag="rowb")
            nc.scalar.activation(lssq[:], s["pssq"][:], AF.Ln,
                                 scale=1.0 / HD, bias=eps_c[0:33, :])
            nc.scalar.activation(rinv_b[:], lssq[:], AF.Exp, scale=-0.5)

            # broadcast rq (rows 0:64) / rk (rows 64:128) over partitions
            prb = psBB.tile([128, CH], FP32, tag="bb")
            nc.tensor.matmul(prb[0:64, :], ones1[0:1, 0:64], rinv_b[0:1, :],
                             start=True, stop=True, skip_group_check=True)
            nc.tensor.matmul(prb[64:128, :], ones1[32:33, 0:64], rinv_b[32:33, :],
                             start=True, stop=True, skip_group_check=True)
            # RMS scale (read straight from PSUM) then RoPE, written into
            # qT / kT halves
            An = tmp.tile([128, CH], BF16, tag="t")
            Bn = tmp.tile([128, CH], BF16, tag="t")
            an_inst = nc.vector.tensor_tensor(out=An[:], in0=s["A_s"][:],
                                              in1=prb[:], op=OP.mult)
            if ch in (1, 2) and "yT_inst" in st[ch - 1]:
                # Force the previous chunk's y^T finalization ahead of this
                # chunk's bulk DVE work: the static scheduler otherwise models
                # the softmax chain as slower than it is and buries the
                # exchange-feeding ops ~15us deep in the DVE stream.
                add_dep_helper(an_inst.ins, st[ch - 1]["yT_inst"].ins,
                               reason="phase2 tail feeds the exchange first")
            nc.vector.tensor_tensor(out=Bn[:], in0=s["B_s"][:], in1=prb[:], op=OP.mult)
            t1 = tmp.tile([128, CH], BF16, tag="t")
            t2 = tmp.tile([128, CH], BF16, tag="t")
            t3 = tmp.tile([128, CH], BF16, tag="t")
            t4 = tmp.tile([128, CH], BF16, tag="t")
            nc.vector.tensor_tensor(out=t1[:], in0=An[:], in1=c2[:, sl], op=OP.mult)
            nc.vector.tensor_tensor(out=t2[:], in0=Bn[:], in1=s2[:, sl], op=OP.mult)
            nc.vector.tensor_tensor(out=t3[:], in0=Bn[:], in1=c2[:, sl], op=OP.mult)
            nc.vector.tensor_tensor(out=t4[:], in0=An[:], in1=s2[:, sl], op=OP.mult)
            nc.vector.tensor_tensor(out=qT[0:64, sl], in0=t1[0:64, :],
                                    in1=t2[0:64, :], op=OP.add)
            nc.vector.tensor_tensor(out=qT[64:128, sl], in0=t3[0:64, :],
                                    in1=t4[0:64, :], op=OP.subtract)
            nc.vector.tensor_tensor(out=kT_t[0:64, sl], in0=t1[64:128, :],
                                    in1=t2[64:128, :], op=OP.add)
            nc.vector.tensor_tensor(out=kT_t[64:128, sl], in0=t3[64:128, :],
                                    in1=t4[64:128, :], op=OP.subtract)

            # v natural: PE-transpose v^T 128x128 tiles, mix with ve in the copy
            for sblk in range(4):
                tt = 4 * ch + sblk
                pv = psM.tile([128, 128], BF16, tag="m")
                nc.tensor.transpose(pv[:], vT[:, tt * 128:(tt + 1) * 128], ident[:])
                nc.vector.tensor_tensor(out=v_sb[:, tt, :], in0=pv[:],
                                        in1=ve[:, tt, :], op=OP.add)

        def phase2(ch):
            sl = slice(ch * CH, (ch + 1) * CH)
            nk = 4 * ch + 4
            py = psY.tile([128, CH], FP32, tag="py")
            pl = psL.tile([128, CH], FP32, tag="pl")
            for ki in range(nk):
                r = ki - 4 * ch  # >=0 on diagonal k-tiles
                lo = max(0, r) * 128  # first valid q column in this chunk
                vs = slice(lo, CH)
                pS = psS.tile([128, CH], FP32, tag="pS")
                nc.tensor.matmul(pS[:, vs], kT_t[:, ki * 128:(ki + 1) * 128],
                                 qT[:, ch * CH + lo:(ch + 1) * CH],
                                 start=True, stop=True)
                pt = ptp.tile([128, CH], BF16, tag="pt")
                nc.scalar.activation(pt[:, vs], pS[:, vs], AF.Exp, scale=ATTN_SCALE)
                if r >= 0:
                    # triangular mask on the single 128-wide diagonal block
                    # (DVE, not gpsimd: gpsimd blocks on in-flight collectives)
                    nc.vector.tensor_tensor(out=pt[:, lo:lo + 128],
                                            in0=pt[:, lo:lo + 128],
                                            in1=tri[:], op=OP.mult)
                nc.tensor.matmul(pl[:, vs], ones128[:], pt[:, vs],
                                 start=(ki == 0), stop=(ki == nk - 1),
                                 skip_group_check=True)
                nc.tensor.matmul(py[:, vs], v_sb[:, ki, :], pt[:, vs],
                                 start=(ki == 0), stop=(ki == nk - 1),
                                 skip_group_check=True)

            # y' = py / ((1 + e^-g) * l); e_g and l arrive pre-broadcast.
            # High priority: yT feeds the exchange, so it must not queue
            # behind the next chunk's bulk DVE work.
            with tc.high_priority(offset=1200):
                den = denp.tile([128, CH], FP32, tag="den")
                rden = denp.tile([128, CH], FP32, tag="rden")
                nc.vector.scalar_tensor_tensor(out=den[:], in0=st[ch]["e_g"][:],
                                               scalar=1.0, in1=pl[:],
                                               op0=OP.add, op1=OP.mult)
                nc.vector.reciprocal_approx_fast(out=rden[:], in_=den[:])
                st[ch]["yT_inst"] = nc.vector.tensor_tensor(
                    out=yT[:, sl], in0=py[:], in1=rden[:], op=OP.mult)

        def stage(ch):
            # The whole exchange pipeline (stage DMA, collective, land DMA)
            # lives on the gpsimd queue: collective n+1 already waits for
            # collective n's completion there, so interleaving the land
            # triggers adds no blocking -- and they fire the moment their
            # exchange completes, instead of queuing behind unrelated sync
            # DMAs (the sync ring interleaves Tile semaphore ops that stall
            # its strict FIFO for tens of us).
            nc.gpsimd.dma_start(
                a2a_in[ch][:].rearrange("(s q) f -> q s f", q=128),
                yT[:, ch * CH:(ch + 1) * CH].rearrange(
                    "p (s f) -> p s f", s=8))
            nc.gpsimd.collective_compute(
                "AllToAll", OP.bypass,
                replica_groups=[list(range(N_CORES))],
                ins=[a2a_in[ch][:].opt()], outs=[a2a_out[ch][:].opt()])

        def land(ch):
            nc.gpsimd.dma_start(
                ygT[:, :, ch, :],
                a2a_out[ch][:].rearrange("(s q) f -> q s f", q=128))

        def outproj(p):
            for oc in range(2):
                po = psS.tile([128, CH], FP32, tag="pS")
                for j in range(8):
                    nc.tensor.matmul(po[:], ygT[:, j, 2 * p:2 * p + 2, :],
                                     wO[:, j, oc * CH:(oc + 1) * CH],
                                     start=(j == 0), stop=(j == 7))
                osb = outp.tile([128, CH], FP32, tag="osb")
                nc.scalar.copy(osb[:], po[:])
                nc.sync.dma_start(
                    out_d[p * 128:(p + 1) * 128, oc * CH:(oc + 1) * CH], osb[:])

        def outproj_half(ch):
            # 64 output rows for one chunk: lets chunk 2's projection run
            # inside the last exchange's wait window
            for oc in range(2):
                po = psS.tile([128, CH], FP32, tag="pS")
                for j in range(8):
                    nc.tensor.matmul(po[0:64, :], ygT[:, j, ch, :],
                                     wO[:, j, oc * CH:(oc + 1) * CH],
                                     start=(j == 0), stop=(j == 7),
                                     skip_group_check=True)
                osb = outp.tile([128, CH], FP32, tag="osb")
                nc.scalar.copy(osb[0:64, :], po[0:64, :])
                nc.sync.dma_start(
                    out_d[ch * 64:(ch + 1) * 64, oc * CH:(oc + 1) * CH],
                    osb[0:64, :])

        p1_mm(0)
        p1_mm(1)
        # W_o only needed for the output projection; load behind the x^T chunks
        nc.sync.dma_start(wO[:], wO_d[:].rearrange("p (i e) -> p i e", i=8))
        p1_fin(0)
        phase2(0)
        stage(0)
        p1_mm(2)
        p1_fin(1)
        phase2(1)
        stage(1)
        land(0)
        p1_mm(3)
        p1_fin(2)
        phase2(2)
        stage(2)
        land(1)
        p1_fin(3)
        phase2(3)
        stage(3)
        land(2)
        land(3)
        outproj(0)      # rows 0..127, runs during the later exchanges
        outproj_half(2)  # rows 128..191, right after exchange 2 lands
        # keep the PE array's clock warm while waiting for the last exchange
        pwu2 = psS.tile([128, CH], FP32, tag="pS")
        for i in range(24):
            nc.tensor.matmul(pwu2[0:1, 0:128], ones[:], ident[:],
                             start=(i == 0), stop=(i == 23),
                             skip_group_check=True)
        outproj_half(3)

    try:
        nc.compile()
    finally:
        ctx_tables.__exit__(None, None, None)
    return nc


def _bf16(a):
    return np.ascontiguousarray(a.astype(ml_dtypes.bfloat16))


def _prep_inputs(x, qkvo_w, gate_w, ve, sa_lambdas, cos, sin):
    x = np.asarray(x, np.float32).reshape(T, D)
    qkvo_w = np.asarray(qkvo_w, np.float32)
    gate_w = np.asarray(gate_w, np.float32)
    ve = np.asarray(ve, np.float32).reshape(T, H, HD)
    sa = np.asarray(sa_lambdas, np.float32)
    cos = np.asarray(cos, np.float32)
    sin = np.asarray(sin, np.float32)

    lam0, lam1 = float(sa[0]), float(sa[1])
    Wq, Wk, Wv, Wo = qkvo_w[0], qkvo_w[1], qkvo_w[2], qkvo_w[3]

    def sb_layout(wT):
        # [D, E] (j-major) -> [128, 8*E]: partition p holds i-tile rows
        E = wT.shape[1]
        return np.ascontiguousarray(
            wT.reshape(8, 128, E).transpose(1, 0, 2).reshape(128, 8 * E))

    xT = _bf16(x.T)                       # [D, T]
    cosT, sinT = cos.T, sin.T             # [64, T]
    c2 = _bf16(np.concatenate([cosT, cosT], 0))   # [128, T]
    s2 = _bf16(np.concatenate([sinT, sinT], 0))
    wO = _bf16(sb_layout(Wo.T))           # [128, 8*D]

    in_maps = []
    for c in range(N_CORES):
        r = slice(c * HD, (c + 1) * HD)
        wq, wk, wv = Wq[r], Wk[r], Wv[r]           # [128, D] each
        wA = _bf16(sb_layout(np.concatenate([wq[0:HALF], wk[0:HALF]], 0).T))
        wB = _bf16(sb_layout(np.concatenate([wq[HALF:], wk[HALF:]], 0).T))
        wVl = _bf16(sb_layout((lam0 * wv).T))
        gwp = np.zeros((128, 1), np.float32)
        gwp[:GATE_IN, 0] = gate_w[c]
        gw128 = np.tile(gwp, (1, 128))             # broadcast gate lhsT
        # ve in [128, KT*HD]: partition p holds rows {p, 128+p, ...}
        ve_c = (lam1 * ve[:, c, :]).reshape(KT, 128, HD).transpose(1, 0, 2)
        in_maps.append({
            "xT": xT, "wA": wA, "wB": wB, "wV": wVl, "wO": wO,
            "c2": c2, "s2": s2,
            "ve_s": _bf16(ve_c.reshape(128, KT * HD)),
            "gw": _bf16(gw128),
        })
    return in_maps


def _profile_hook():
    so_path = "/opt/axon/libaxon_pjrt.so"
    lib = ctypes.CDLL(so_path)
    if not hasattr(lib, "axon_start_nrt_profile"):
        return None
    lib.axon_start_nrt_profile.argtypes = [ctypes.POINTER(ctypes.c_int64),
                                           ctypes.c_size_t]
    lib.axon_start_nrt_profile.restype = ctypes.c_int64
    lib.axon_stop_nrt_profile.argtypes = [ctypes.c_char_p]
    lib.axon_stop_nrt_profile.restype = ctypes.c_int64

    @contextlib.contextmanager
    def _hook(output_dir, device_ids):
        import jax
        jax.devices()
        if device_ids:
            ids = (ctypes.c_int64 * len(device_ids))(*device_ids)
            rc = lib.axon_start_nrt_profile(ids, len(device_ids))
        else:
            rc = lib.axon_start_nrt_profile(None, 0)
        if rc != 0:
            raise RuntimeError(f"axon_start_nrt_profile rc={rc}")
        try:
            yield
        finally:
            n = lib.axon_stop_nrt_profile(str(output_dir).encode())
            print(f"profile: {n} file(s) -> {output_dir}", file=sys.stderr)

    return _hook


def _maybe_enable_profiling():
    if os.environ.get("KERNEL_PROFILE") != "1":
        return False
    try:
        hook = _profile_hook()
        if hook is None:
            return False
        mod = types.ModuleType("antenv.axon_hooks")
        mod.get_axon_ntff_profile_hook = lambda: hook
        sys.modules["antenv.axon_hooks"] = mod
        bass_utils.upload_artifacts = lambda tmpdir: tmpdir
        return True
    except Exception as e:  # profiling is best-effort
        print(f"profiling unavailable: {e}", file=sys.stderr)
        return False


def kernel(x, qkvo_w, gate_w, ve, sa_lambdas, cos, sin):
    in_maps = _prep_inputs(x, qkvo_w, gate_w, ve, sa_lambdas, cos, sin)
    nc = _build_program()
    trace = _maybe_enable_profiling()
    res = bass_utils.run_bass_kernel_spmd(
        nc, in_maps, core_ids=list(range(N_CORES)), trace=trace)
    LAST_RUN_INFO["exec_time_ns"] = res.exec_time_ns
    LAST_RUN_INFO["profile_json"] = res.profile_json

    # core c's out_t row (ch*64 + i) is global t = 512*ch + 64*c + i
    out = np.empty((T, D), np.float32)
    for c in range(N_CORES):
        rows = res.results[c]["out_t"]
        for ch in range(NCH):
            t0 = CH * ch + BLK * c
            out[t0:t0 + BLK] = rows[ch * BLK:(ch + 1) * BLK]
    return out.reshape(1, T, D)


# revision 36
# speedup vs baseline: 1.0114x; 1.0114x over previous
"""Trainium2 Bass kernel for nn_CausalSelfAttention_2224793059575.

Tensor-parallel over heads across 8 NeuronCores: core c owns head c
(B=1, T=2048, D=1024, H=8, HD=128). Per core:

  - QKV projection (contraction over D) consumes x^T (host-prepared layout,
    bf16) against per-head weight slices, emitting q/k in a transposed
    [head_dim, T] layout stacked as A=[q_lo;k_lo], B=[q_hi;k_hi] so that
    RMS-norm scaling and RoPE run as full-128-partition DVE ops.
  - Each 512-col chunk is split into a matmul-only part (p1_mm: QKV, gate
    logits, sum-of-squares) and a dependent finish (p1_fin: rsqrt chain,
    RMS scale, RoPE written straight into qT/kT halves, v transpose), so
    the PE never sits behind the ACT/DVE chain: p1_fin(ch) is emitted
    after p1_mm(ch+1).
  - The QKV PSUM accumulators are copied to SBUF (bf16) immediately, so
    two PSUM banks suffice for double-buffered QKV.
  - rsqrt via Ln/Exp on ScalarE. All ACT functions used (exp/ln/square/
    copy) live in the natural_log_exp_and_others table set; the table list
    handed to the load-insertion pass is filtered so that set is the only
    candidate, giving exactly one ACT_TABLE_LOAD for the whole kernel
    (walrus otherwise alternates exp_and_others <-> natural_log, a 3us
    reload per chunk on the critical path).
  - Scores are computed transposed (S^T[k,q]); exp on ScalarE; causal
    masking only of the 128x128 diagonal block; softmax denominator via an
    all-ones [128,128] lhsT so the row-sums arrive pre-broadcast over all
    partitions; the head-gate logits use a host-tiled [128,128] gw so the
    sigmoid input is also pre-broadcast. The normalization
    y' = y / ((1+e^-g) * l) is then 3 wide DVE ops, no PE broadcasts.
  - y^T chunks are exchanged with four 128KB AllToAlls, one per chunk,
    triggered as soon as each chunk's y^T is finalized. The entire exchange
    pipeline (stage DMA -> collective -> land DMA) lives on the gpsimd DMA
    ring: collective n+1 already waits for collective n's completion there,
    so the land triggers add no blocking and fire the moment their exchange
    completes (the sync ring's strict FIFO interleaves Tile semaphore ops
    that would stall them for tens of us). A full-size garbage AllToAll on
    the same buffers runs during the compute fill phase to absorb the
    kernel-start skew and the ~20-25us first-collective channel-wake cost,
    after which each real exchange takes ~8-14us. The output projection for
    rows 0..127 and chunk 2 runs inside the later exchanges' wait windows;
    chunk 3's rows follow the last landing. Host reassembles the
    interleaved row blocks.

Sharding/layout prep (slicing qkvo_w per head, transposes, bf16 casts,
folding sa_lambdas into the weight slices) happens host-side in numpy, as
input preparation; all FLOPs of the module run on the NeuronCores.
"""
import contextlib
import ctypes
import os
import sys
import types

import numpy as np

for _p in ("/opt/trn_rl_repo",):
    if _p not in sys.path:
        sys.path.append(_p)

import ml_dtypes  # noqa: E402

import concourse.bacc as bacc  # noqa: E402
import concourse.mybir as mybir  # noqa: E402
import concourse.tile as tile  # noqa: E402
from concourse import bass_utils  # noqa: E402
from concourse.tile_rust import add_dep_helper  # noqa: E402

BF16 = mybir.dt.bfloat16
FP32 = mybir.dt.float32
AF = mybir.ActivationFunctionType
OP = mybir.AluOpType

N_CORES = 8
T = 2048
D = 1024
H = 8
HD = 128
HALF = HD // 2  # 64
NCH = 4          # T chunks of 512
CH = T // NCH    # 512
KT = T // 128    # 16 k-tiles
BLK = CH // N_CORES  # 64-wide t-blocks for the interleaved A2A sharding
ATTN_SCALE = 0.12
EPS = 1e-6
GATE_IN = 12

LAST_RUN_INFO = {}


@contextlib.contextmanager
def _single_act_table():
    """Restrict the ACT table-set candidates to natural_log_exp_and_others.

    The set genuinely contains every function this kernel uses (exp, ln,
    square, copy/identity), but the load-insertion pass otherwise picks the
    first set containing each function (exp_and_others for exp, natural_log
    for ln), forcing a ~1.5us table reload at every ln<->exp transition.
    Names and positions are preserved so the emitted act_func_set_id still
    matches act_info.json.
    """
    orig = bacc.get_activation_tables

    def patched(arch):
        tabs = orig(arch)
        return {
            name: (fns if name == "natural_log_exp_and_others" else set())
            for name, fns in tabs.items()
        }

    bacc.get_activation_tables = patched
    try:
        yield
    finally:
        bacc.get_activation_tables = orig


def _build_program():
    nc = bacc.Bacc("TRN2", target_bir_lowering=False, debug=False,
                   num_devices=N_CORES)
    ctx_tables = _single_act_table()
    ctx_tables.__enter__()

    # ---- kernel I/O ----
    xT_d = nc.dram_tensor("xT", [D, T], BF16, kind="ExternalInput")
    wA_d = nc.dram_tensor("wA", [128, 8 * 128], BF16, kind="ExternalInput")
    wB_d = nc.dram_tensor("wB", [128, 8 * 128], BF16, kind="ExternalInput")
    wV_d = nc.dram_tensor("wV", [128, 8 * 128], BF16, kind="ExternalInput")
    wO_d = nc.dram_tensor("wO", [128, 8 * D], BF16, kind="ExternalInput")
    c2_d = nc.dram_tensor("c2", [128, T], BF16, kind="ExternalInput")
    s2_d = nc.dram_tensor("s2", [128, T], BF16, kind="ExternalInput")
    ve_d = nc.dram_tensor("ve_s", [128, KT * HD], BF16, kind="ExternalInput")
    gw_d = nc.dram_tensor("gw", [128, 128], BF16, kind="ExternalInput")
    out_d = nc.dram_tensor("out_t", [T // N_CORES, D], FP32, kind="ExternalOutput")

    with tile.TileContext(nc) as tc, contextlib.ExitStack() as ctx:
        P = ctx.enter_context

        cons = P(tc.tile_pool(name="cons", bufs=1))
        work = P(tc.tile_pool(name="work", bufs=1))
        absp = P(tc.tile_pool(name="absp", bufs=4))
        sqp = P(tc.tile_pool(name="sqp", bufs=4))
        rowp = P(tc.tile_pool(name="rowp", bufs=2))
        rbp = P(tc.tile_pool(name="rbp", bufs=2))
        tmp = P(tc.tile_pool(name="tmp", bufs=8))
        egp = P(tc.tile_pool(name="egp", bufs=3))
        ptp = P(tc.tile_pool(name="ptp", bufs=14))
        denp = P(tc.tile_pool(name="denp", bufs=3))
        outp = P(tc.tile_pool(name="outp", bufs=2))
        dram = P(tc.tile_pool(name="dram", bufs=1, space="DRAM"))

        # PSUM: 8 banks total, statically budgeted
        psAB = P(tc.tile_pool(name="psAB", bufs=2, space="PSUM"))
        psS = P(tc.tile_pool(name="psS", bufs=2, space="PSUM"))
        psY = P(tc.tile_pool(name="psY", bufs=1, space="PSUM"))
        psBB = P(tc.tile_pool(name="psBB", bufs=1, space="PSUM"))
        psM = P(tc.tile_pool(name="psM", bufs=1, space="PSUM"))
        psL = P(tc.tile_pool(name="psL", bufs=1, space="PSUM"))

        # ---- persistent SBUF ----
        xT = cons.tile([128, 8, T], BF16)          # x^T, i-tile major
        wA = cons.tile([128, 8, 128], BF16)
        wB = cons.tile([128, 8, 128], BF16)
        wV = cons.tile([128, 8, 128], BF16)
        wO = cons.tile([128, 8, D], BF16)
        c2 = cons.tile([128, T], BF16)
        s2 = cons.tile([128, T], BF16)
        ve = cons.tile([128, KT, HD], BF16)
        gw = cons.tile([128, 128], BF16)
        ones = cons.tile([128, 1], BF16)
        ones128 = cons.tile([128, 128], BF16)
        ones1 = cons.tile([33, 128], BF16)
        sel33 = cons.tile([128, 33], BF16)
        ident = cons.tile([128, 128], BF16)
        tri = cons.tile([128, 128], BF16)
        eps_c = cons.tile([128, 1], FP32)
        wup = cons.tile([128, 512], BF16)  # zeroed warmup operand

        qT = work.tile([128, T], BF16)
        kT_t = work.tile([128, T], BF16)
        vT = work.tile([128, T], BF16)
        v_sb = work.tile([128, KT, HD], BF16)
        yT = work.tile([128, T], BF16)
        # [j-tile, ch, 64] free layout; [:, j, 2p:2p+2, :] is a contiguous
        # 128-wide lhsT slice for the output projection
        ygT = work.tile([128, 8, NCH, BLK], BF16)

        a2a_in = [dram.tile([D, BLK], BF16, name=f"a2ain{i}") for i in range(NCH)]
        a2a_out = [dram.tile([D, BLK], BF16, name=f"a2aout{i}") for i in range(NCH)]
        bar_in = dram.tile([8, 64], BF16, name="barin")
        bar_out = dram.tile([8, 64], BF16, name="barout")

        # PE warmup first thing: result is never read -- it exists purely to
        # get HAM to 8/8 before the first QKV matmul. wup is zeroed on the
        # otherwise-idle DVE so the warmup doesn't wait for gpsimd's memsets.
        nc.vector.memset(wup[:], 0.0)
        pwu = psS.tile([128, CH], FP32, tag="pS")
        # N=512 warmup matmuls: ~6-9us of sustained PE activity so the HAM
        # clock-gate stays at 8/8 across the input-DMA window until the first
        # QKV matmul is ready (a >3.4us idle gap re-throttles to 4/8).
        for i in range(12):
            nc.tensor.matmul(pwu[0:1, :], wup[:, 0:1], wup[:],
                             start=(i == 0), stop=(i == 11),
                             skip_group_check=True)

        # ---- on-chip constants + priority-ordered input DMAs ----
        # Constant setup runs on the otherwise-idle DVE; gpsimd keeps only
        # the two affine_selects (gpsimd-only op), so the warm collective
        # below fires within ~2us of kernel start.
        nc.vector.memset(ident[:], 1.0)
        nc.vector.memset(tri[:], 1.0)
        # keep 1.0 where p == f, else 0
        nc.gpsimd.affine_select(out=ident[:], in_=ident[:], compare_op=OP.is_equal,
                                fill=0.0, base=0, pattern=[[-1, 128]],
                                channel_multiplier=1)
        # upper-triangular (incl. diagonal) ones: keep where f - p >= 0
        nc.gpsimd.affine_select(out=tri[:], in_=tri[:], compare_op=OP.is_ge,
                                fill=0.0, base=0, pattern=[[1, 128]],
                                channel_multiplier=-1)

        # Warm the collective path end-to-end with a full-size garbage
        # exchange on the real buffers: the first data-size collective of a
        # run pays ~20-25us of one-time channel wake/init plus the
        # kernel-start skew across cores, and the exchange chain is gated on
        # its completion -- every us earlier here is a us off the chain head.
        nc.gpsimd.collective_compute(
            "AllToAll", OP.bypass,
            replica_groups=[list(range(N_CORES))],
            ins=[a2a_in[0][:].opt()], outs=[a2a_out[0][:].opt()])

        nc.vector.memset(ones[:], 1.0)
        nc.vector.memset(ones128[:], 1.0)
        nc.vector.memset(ones1[:], 1.0)
        nc.vector.memset(sel33[:], 0.0)
        nc.vector.memset(sel33[0:64, 0:1], 1.0)
        nc.vector.memset(sel33[64:128, 32:33], 1.0)
        nc.vector.memset(eps_c[:], EPS)

        nc.sync.dma_start(wA[:], wA_d[:].rearrange("p (i e) -> p i e", i=8))
        nc.sync.dma_start(gw[:], gw_d[:])
        for i in range(8):
            nc.sync.dma_start(xT[:, i, 0:CH], xT_d[i * 128:(i + 1) * 128, 0:CH])
        nc.sync.dma_start(wB[:], wB_d[:].rearrange("p (i e) -> p i e", i=8))
        nc.sync.dma_start(wV[:], wV_d[:].rearrange("p (i e) -> p i e", i=8))
        for i in range(8):
            nc.sync.dma_start(xT[:, i, CH:2 * CH],
                              xT_d[i * 128:(i + 1) * 128, CH:2 * CH])
        for i in range(8):
            nc.sync.dma_start(xT[:, i, 2 * CH:T],
                              xT_d[i * 128:(i + 1) * 128, 2 * CH:T])
        nc.sync.dma_start(c2[:], c2_d[:])
        nc.sync.dma_start(s2[:], s2_d[:])
        nc.sync.dma_start(ve[:], ve_d[:].rearrange("p (k d) -> p k d", k=KT))

        # per-chunk state handed from p1_mm to p1_fin / phase2
        st = {}

        def p1_mm(ch):
            sl = slice(ch * CH, (ch + 1) * CH)
            s = {}
            # QKV (A/B stacked halves of q,k; v transposed), N=512 matmuls
            pA = psAB.tile([128, CH], FP32, tag="pAB")
            pB = psAB.tile([128, CH], FP32, tag="pAB")
            for i in range(8):
                nc.tensor.matmul(pA[:], wA[:, i, :], xT[:, i, sl],
                                 start=(i == 0), stop=(i == 7))
            # free the PSUM bank quickly: bf16 copy on DVE, square on ACT
            # (parallel engines keep the rsqrt chain short)
            A_s = absp.tile([128, CH], BF16, tag="ab_sb")
            sqA = sqp.tile([128, CH], BF16, tag="sq")
            with tc.high_priority(offset=1200):
                nc.vector.tensor_copy(out=A_s[:], in_=pA[:])
            nc.scalar.activation(sqA[:], pA[:], AF.Square)

            # gate logits, broadcast over all partitions via host-tiled gw;
            # emitted between the A and B blocks so e_g's ACT read of the psM
            # bank finishes under the B matmuls (pVt reuses that bank)
            pg = psM.tile([128, CH], FP32, tag="m")
            nc.tensor.matmul(pg[:], gw[:], xT[:, 0, sl], start=True, stop=True)
            e_g = egp.tile([128, CH], BF16, tag="eg")
            nc.scalar.activation(e_g[:], pg[:], AF.Exp, scale=-1.0)
            s["e_g"] = e_g

            for i in range(8):
                nc.tensor.matmul(pB[:], wB[:, i, :], xT[:, i, sl],
                                 start=(i == 0), stop=(i == 7))
            B_s = absp.tile([128, CH], BF16, tag="ab_sb")
            sqB = sqp.tile([128, CH], BF16, tag="sq")
            with tc.high_priority(offset=1200):
                nc.vector.tensor_copy(out=B_s[:], in_=pB[:])
            nc.scalar.activation(sqB[:], pB[:], AF.Square)

            pVt = psM.tile([128, CH], FP32, tag="m")
            for i in range(8):
                nc.tensor.matmul(pVt[:], wV[:, i, :], xT[:, i, sl],
                                 start=(i == 0), stop=(i == 7))
            nc.vector.tensor_copy(out=vT[:, sl], in_=pVt[:])

            # sum of squares -> [33, CH] rows (q at partition 0; k at 32)
            pssq = psBB.tile([33, CH], FP32, tag="bb")
            nc.tensor.matmul(pssq[:], sel33[:], sqA[:], start=True, stop=False)
            nc.tensor.matmul(pssq[:], sel33[:], sqB[:], start=False, stop=True)
            s["pssq"] = pssq
            s["A_s"] = A_s
            s["B_s"] = B_s
            st[ch] = s

        def p1_fin(ch):
            sl = slice(ch * CH, (ch + 1) * CH)
            s = st[ch]
            # rsqrt rows via Ln/Exp (one shared ACT table set)
            lssq = rowp.tile([33, CH], FP32, tag="row")
            rinv_b = rowp.tile([33, CH], BF16, tag="rowb")
            nc.scalar.activation(lssq[:], s["pssq"][:], AF.Ln,
                                 scale=1.0 / HD, bias=eps_c[0:33, :])
            nc.scalar.activation(rinv_b[:], lssq[:], AF.Exp, scale=-0.5)

            # broadcast rq (rows 0:64) / rk (rows 64:128) over partitions
            prb = psBB.tile([128, CH], FP32, tag="bb")
            nc.tensor.matmul(prb[0:64, :], ones1[0:1, 0:64], rinv_b[0:1, :],
                             start=True, stop=True, skip_group_check=True)
            nc.tensor.matmul(prb[64:128, :], ones1[32:33, 0:64], rinv_b[32:33, :],
                             start=True, stop=True, skip_group_check=True)
            # RMS scale (read straight from PSUM) then RoPE, written into
            # qT / kT halves
            An = tmp.tile([128, CH], BF16, tag="t")
            Bn = tmp.tile([128, CH], BF16, tag="t")
            an_inst = nc.vector.tensor_tensor(out=An[:], in0=s["A_s"][:],
                                              in1=prb[:], op=OP.mult)
            if ch in (1, 2) and "yT_inst" in st[ch - 1]:
                # Force the previous chunk's y^T finalization ahead of this
                # chunk's bulk DVE work: the static scheduler otherwise models
                # the softmax chain as slower than it is and buries the
                # exchange-feeding ops ~15us deep in the DVE stream.
                add_dep_helper(an_inst.ins, st[ch - 1]["yT_inst"].ins,
                               reason="phase2 tail feeds the exchange first")
            nc.vector.tensor_tensor(out=Bn[:], in0=s["B_s"][:], in1=prb[:], op=OP.mult)
            t1 = tmp.tile([128, CH], BF16, tag="t")
            t2 = tmp.tile([128, CH], BF16, tag="t")
            t3 = tmp.tile([128, CH], BF16, tag="t")
            t4 = tmp.tile([128, CH], BF16, tag="t")
            nc.vector.tensor_tensor(out=t1[:], in0=An[:], in1=c2[:, sl], op=OP.mult)
            nc.vector.tensor_tensor(out=t2[:], in0=Bn[:], in1=s2[:, sl], op=OP.mult)
            nc.vector.tensor_tensor(out=t3[:], in0=Bn[:], in1=c2[:, sl], op=OP.mult)
            nc.vector.tensor_tensor(out=t4[:], in0=An[:], in1=s2[:, sl], op=OP.mult)
            nc.vector.tensor_tensor(out=qT[0:64, sl], in0=t1[0:64, :],
                                    in1=t2[0:64, :], op=OP.add)
            nc.vector.tensor_tensor(out=qT[64:128, sl], in0=t3[0:64, :],
                                    in1=t4[0:64, :], op=OP.subtract)
            nc.vector.tensor_tensor(out=kT_t[0:64, sl], in0=t1[64:128, :],
                                    in1=t2[64:128, :], op=OP.add)
            nc.vector.tensor_tensor(out=kT_t[64:128, sl], in0=t3[64:128, :],
                                    in1=t4[64:128, :], op=OP.subtract)

            # v natural: PE-transpose v^T 128x128 tiles, mix with ve in the copy
            for sblk in range(4):
                tt = 4 * ch + sblk
                pv = psM.tile([128, 128], BF16, tag="m")
                nc.tensor.transpose(pv[:], vT[:, tt * 128:(tt + 1) * 128], ident[:])
                nc.vector.tensor_tensor(out=v_sb[:, tt, :], in0=pv[:],
                                        in1=ve[:, tt, :], op=OP.add)

        def phase2(ch):
            sl = slice(ch * CH, (ch + 1) * CH)
            nk = 4 * ch + 4
            py = psY.tile([128, CH], FP32, tag="py")
            pl = psL.tile([128, CH], FP32, tag="pl")
            for ki in range(nk):
                r = ki - 4 * ch  # >=0 on diagonal k-tiles
                lo = max(0, r) * 128  # first valid q column in this chunk
                vs = slice(lo, CH)
                pS = psS.tile([128, CH], FP32, tag="pS")
                nc.tensor.matmul(pS[:, vs], kT_t[:, ki * 128:(ki + 1) * 128],
                                 qT[:, ch * CH + lo:(ch + 1) * CH],
                                 start=True, stop=True)
                pt = ptp.tile([128, CH], BF16, tag="pt")
                nc.scalar.activation(pt[:, vs], pS[:, vs], AF.Exp, scale=ATTN_SCALE)
                if r >= 0:
                    # triangular mask on the single 128-wide diagonal block
                    # (DVE, not gpsimd: gpsimd blocks on in-flight collectives)
                    nc.vector.tensor_tensor(out=pt[:, lo:lo + 128],
                                            in0=pt[:, lo:lo + 128],
                                            in1=tri[:], op=OP.mult)
                nc.tensor.matmul(pl[:, vs], ones128[:], pt[:, vs],
                                 start=(ki == 0), stop=(ki == nk - 1),
                                 skip_group_check=True)
                nc.tensor.matmul(py[:, vs], v_sb[:, ki, :], pt[:, vs],
                                 start=(ki == 0), stop=(ki == nk - 1),
                                 skip_group_check=True)

            # y' = py / ((1 + e^-g) * l); e_g and l arrive pre-broadcast.
            # High priority: yT feeds the exchange, so it must not queue
            # behind the next chunk's bulk DVE work.
            with tc.high_priority(offset=1200):
                den = denp.tile([128, CH], FP32, tag="den")
                rden = denp.tile([128, CH], FP32, tag="rden")
                nc.vector.scalar_tensor_tensor(out=den[:], in0=st[ch]["e_g"][:],
                                               scalar=1.0, in1=pl[:],
                                               op0=OP.add, op1=OP.mult)
                nc.vector.reciprocal_approx_fast(out=rden[:], in_=den[:])
                st[ch]["yT_inst"] = nc.vector.tensor_tensor(
                    out=yT[:, sl], in0=py[:], in1=rden[:], op=OP.mult)

        def stage(ch):
            # The whole exchange pipeline (stage DMA, collective, land DMA)
            # lives on the gpsimd queue: collective n+1 already waits for
            # collective n's completion there, so interleaving the land
            # triggers adds no blocking -- and they fire the moment their
            # exchange completes, instead of queuing behind unrelated sync
            # DMAs (the sync ring interleaves Tile semaphore ops that stall
            # its strict FIFO for tens of us).
            nc.gpsimd.dma_start(
                a2a_in[ch][:].rearrange("(s q) f -> q s f", q=128),
                yT[:, ch * CH:(ch + 1) * CH].rearrange(
                    "p (s f) -> p s f", s=8))
            nc.gpsimd.collective_compute(
                "AllToAll", OP.bypass,
                replica_groups=[list(range(N_CORES))],
                ins=[a2a_in[ch][:].opt()], outs=[a2a_out[ch][:].opt()])

        def land(ch):
            nc.gpsimd.dma_start(
                ygT[:, :, ch, :],
                a2a_out[ch][:].rearrange("(s q) f -> q s f", q=128))

        def outproj(p):
            for oc in range(2):
                po = psS.tile([128, CH], FP32, tag="pS")
                for j in range(8):
                    nc.tensor.matmul(po[:], ygT[:, j, 2 * p:2 * p + 2, :],
                                     wO[:, j, oc * CH:(oc + 1) * CH],
                                     start=(j == 0), stop=(j == 7))
                osb = outp.tile([128, CH], FP32, tag="osb")
                nc.scalar.copy(osb[:], po[:])
                nc.sync.dma_start(
                    out_d[p * 128:(p + 1) * 128, oc * CH:(oc + 1) * CH], osb[:])

        def outproj_half(ch):
            # 64 output rows for one chunk: lets chunk 2's projection run
            # inside the last exchange's wait window
            for oc in range(2):
                po = psS.tile([128, CH], FP32, tag="pS")
                for j in range(8):
                    nc.tensor.matmul(po[0:64, :], ygT[:, j, ch, :],
                                     wO[:, j, oc * CH:(oc + 1) * CH],
                                     start=(j == 0), stop=(j == 7),
                                     skip_group_check=True)
                osb = outp.tile([128, CH], FP32, tag="osb")
                nc.scalar.copy(osb[0:64, :], po[0:64, :])
                nc.sync.dma_start(
                    out_d[ch * 64:(ch + 1) * 64, oc * CH:(oc + 1) * CH],
                    osb[0:64, :])

        p1_mm(0)
        p1_mm(1)
        # W_o only needed for the output projection; load behind the x^T chunks
        nc.sync.dma_start(wO[:], wO_d[:].rearrange("p (i e) -> p i e", i=8))
        p1_fin(0)
        phase2(0)
        stage(0)
        p1_mm(2)
        p1_fin(1)
        phase2(1)
        stage(1)
        land(0)
        p1_mm(3)
        p1_fin(2)
        phase2(2)
        stage(2)
        land(1)
        p1_fin(3)
        phase2(3)
        stage(3)
        land(2)
        land(3)
        outproj(0)      # rows 0..127, runs during the later exchanges
        outproj_half(2)  # rows 128..191, right after exchange 2 lands
        # keep the PE array's clock warm while waiting for the last exchange
        pwu2 = psS.tile([128, CH], FP32, tag="pS")
        for i in range(20):
            nc.tensor.matmul(pwu2[0:1, :], wup[:, 0:1], wup[:],
                             start=(i == 0), stop=(i == 19),
                             skip_group_check=True)
        outproj_half(3)

    try:
        nc.compile()
    finally:
        ctx_tables.__exit__(None, None, None)
    return nc


def _bf16(a):
    return np.ascontiguousarray(a.astype(ml_dtypes.bfloat16))


def _prep_inputs(x, qkvo_w, gate_w, ve, sa_lambdas, cos, sin):
    x = np.asarray(x, np.float32).reshape(T, D)
    qkvo_w = np.asarray(qkvo_w, np.float32)
    gate_w = np.asarray(gate_w, np.float32)
    ve = np.asarray(ve, np.float32).reshape(T, H, HD)
    sa = np.asarray(sa_lambdas, np.float32)
    cos = np.asarray(cos, np.float32)
    sin = np.asarray(sin, np.float32)

    lam0, lam1 = float(sa[0]), float(sa[1])
    Wq, Wk, Wv, Wo = qkvo_w[0], qkvo_w[1], qkvo_w[2], qkvo_w[3]

    def sb_layout(wT):
        # [D, E] (j-major) -> [128, 8*E]: partition p holds i-tile rows
        E = wT.shape[1]
        return np.ascontiguousarray(
            wT.reshape(8, 128, E).transpose(1, 0, 2).reshape(128, 8 * E))

    xT = _bf16(x.T)                       # [D, T]
    cosT, sinT = cos.T, sin.T             # [64, T]
    c2 = _bf16(np.concatenate([cosT, cosT], 0))   # [128, T]
    s2 = _bf16(np.concatenate([sinT, sinT], 0))
    wO = _bf16(sb_layout(Wo.T))           # [128, 8*D]

    in_maps = []
    for c in range(N_CORES):
        r = slice(c * HD, (c + 1) * HD)
        wq, wk, wv = Wq[r], Wk[r], Wv[r]           # [128, D] each
        wA = _bf16(sb_layout(np.concatenate([wq[0:HALF], wk[0:HALF]], 0).T))
        wB = _bf16(sb_layout(np.concatenate([wq[HALF:], wk[HALF:]], 0).T))
        wVl = _bf16(sb_layout((lam0 * wv).T))
        gwp = np.zeros((128, 1), np.float32)
        gwp[:GATE_IN, 0] = gate_w[c]
        gw128 = np.tile(gwp, (1, 128))             # broadcast gate lhsT
        # ve in [128, KT*HD]: partition p holds rows {p, 128+p, ...}
        ve_c = (lam1 * ve[:, c, :]).reshape(KT, 128, HD).transpose(1, 0, 2)
        in_maps.append({
            "xT": xT, "wA": wA, "wB": wB, "wV": wVl, "wO": wO,
            "c2": c2, "s2": s2,
            "ve_s": _bf16(ve_c.reshape(128, KT * HD)),
            "gw": _bf16(gw128),
        })
    return in_maps


def _profile_hook():
    so_path = "/opt/axon/libaxon_pjrt.so"
    lib = ctypes.CDLL(so_path)
    if not hasattr(lib, "axon_start_nrt_profile"):
        return None
    lib.axon_start_nrt_profile.argtypes = [ctypes.POINTER(ctypes.c_int64),
                                           ctypes.c_size_t]
    lib.axon_start_nrt_profile.restype = ctypes.c_int64
    lib.axon_stop_nrt_profile.argtypes = [ctypes.c_char_p]
    lib.axon_stop_nrt_profile.restype = ctypes.c_int64

    @contextlib.contextmanager
    def _hook(output_dir, device_ids):
        import jax
        jax.devices()
        if device_ids:
            ids = (ctypes.c_int64 * len(device_ids))(*device_ids)
            rc = lib.axon_start_nrt_profile(ids, len(device_ids))
        else:
            rc = lib.axon_start_nrt_profile(None, 0)
        if rc != 0:
            raise RuntimeError(f"axon_start_nrt_profile rc={rc}")
        try:
            yield
        finally:
            n = lib.axon_stop_nrt_profile(str(output_dir).encode())
            print(f"profile: {n} file(s) -> {output_dir}", file=sys.stderr)

    return _hook


def _maybe_enable_profiling():
    if os.environ.get("KERNEL_PROFILE") != "1":
        return False
    try:
        hook = _profile_hook()
        if hook is None:
            return False
        mod = types.ModuleType("antenv.axon_hooks")
        mod.get_axon_ntff_profile_hook = lambda: hook
        sys.modules["antenv.axon_hooks"] = mod
        bass_utils.upload_artifacts = lambda tmpdir: tmpdir
        return True
    except Exception as e:  # profiling is best-effort
        print(f"profiling unavailable: {e}", file=sys.stderr)
        return False


def kernel(x, qkvo_w, gate_w, ve, sa_lambdas, cos, sin):
    in_maps = _prep_inputs(x, qkvo_w, gate_w, ve, sa_lambdas, cos, sin)
    nc = _build_program()
    trace = _maybe_enable_profiling()
    res = bass_utils.run_bass_kernel_spmd(
        nc, in_maps, core_ids=list(range(N_CORES)), trace=trace)
    LAST_RUN_INFO["exec_time_ns"] = res.exec_time_ns
    LAST_RUN_INFO["profile_json"] = res.profile_json

    # core c's out_t row (ch*64 + i) is global t = 512*ch + 64*c + i
    out = np.empty((T, D), np.float32)
    for c in range(N_CORES):
        rows = res.results[c]["out_t"]
        for ch in range(NCH):
            t0 = CH * ch + BLK * c
            out[t0:t0 + BLK] = rows[ch * BLK:(ch + 1) * BLK]
    return out.reshape(1, T, D)
